# revision 20
# baseline (speedup 1.0000x reference)
"""Trainium2 Bass kernel for nn_OmegaEntangle (E^T C E with entangle coefficients).

Math (validated vs reference to ~5.3e-3 rel err in the numpy bf16 model):
  p_i = sum_j v_ij^2 ; m_i = mean_j v_ij
  C[i,j] = mask(i<j) * sqrt(p_i p_j) * (m_i + 1j*m_j) / sqrt(m_i^2 + m_j^2)
  out = E^T C E   (complex, E real)

Factorization used on device (amp factors folded into operand scaling):
  G[i,j]  = mask(i<j) / sqrt(m_i^2 + m_j^2)          (the only matrix built on-chip)
  T_re    = diag(a) G (diag(sp) E) ;  T_im = diag(sp) G (diag(a) E)
  out_re  = E^T T_re ; out_im = E^T T_im             (a = m*sqrt(p), sp = sqrt(p))

Sharding: data-parallel over the 2048 OUTPUT COLUMNS (256 per core), with the
p/m reduction row-sharded (64 rows per core => 128 SBUF partitions).

Two NEFF launches (host concat of the tiny reduction result between them):
  Kernel A: 2-engine reduce of the [128, 16384] bf16 vuln shard:
    Act does Square+accum on 7 chunks; DVE squares 1 chunk and computes the
    plain sum via a pairwise tensor_tensor tree (2x bf16) + one tensor_reduce.
  Kernel B: build G via Abs_reciprocal_sqrt + mask, two bf16 matmul chains,
    write transposed [256, 2048] bf16 slabs for re/im.
DMA layouts use >=4KB per-partition contiguous runs (small packets starve the
DMA engines). Only HWDGE queues (sync/scalar) carry bulk data; the gating small
tensors are ordered ahead of the 2MB E stream so they are not starved, and the
e2 bulk rides behind the masks on sync. PE warm-up fillers run before chain1 so
the HAM clock reaches 2.4GHz without a >3.4us idle re-throttle.
"""

import numpy as np
import ml_dtypes

import concourse.bass as bass
import concourse.mybir as mybir
import concourse.tile as tile
from concourse import bacc
from concourse.bass_utils import run_bass_kernel_spmd

D = 512          # number of domains
V = 32768        # vuln dim
S = 2048         # sup (embed) dim
NCORES = 8
ROWS_PER_CORE = D // NCORES          # 64
COLS_PER_CORE = S // NCORES          # 256
KT = D // 128                         # 4 contraction tiles
VPART = (ROWS_PER_CORE * V) // 128    # 16384 vuln elems per partition
NCH = 8                               # reduce chunks per core
CH = VPART // NCH                     # 2048
WARMUP_MMS = 5

F32 = mybir.dt.float32
BF16 = mybir.dt.bfloat16
NP_BF16 = ml_dtypes.bfloat16
AF = mybir.ActivationFunctionType
ALU = mybir.AluOpType

_CACHE = {}


def build_kernel_a():
    """Reduce kernel: per-partition p/msum over the [128, 16384] bf16 shard."""
    nc = bacc.Bacc("TRN2", target_bir_lowering=False, debug=False, num_devices=NCORES)

    v128 = nc.dram_tensor("v128", [128, VPART], BF16, kind="ExternalInput")
    out_pm = nc.dram_tensor("out_pm", [128, NCH], F32, kind="ExternalOutput")
    out_ms = nc.dram_tensor("out_ms", [128, CH // 2], BF16,
                            kind="ExternalOutput")

    with tile.TileContext(nc) as tc:
        with (
            tc.tile_pool(name="vin", bufs=NCH) as vin_pool,
            tc.tile_pool(name="scrA", bufs=3) as scrA_pool,
            tc.tile_pool(name="scrD", bufs=2) as scrD_pool,
            tc.tile_pool(name="tree", bufs=4) as tree_pool,
            tc.tile_pool(name="small", bufs=1) as small_pool,
        ):
            vts = []
            qmap = {0: nc.sync, 1: nc.scalar, 2: nc.sync, 3: nc.scalar,
                    4: nc.sync, 5: nc.scalar, 6: nc.sync, 7: nc.scalar}
            for t in range(NCH):
                vt = vin_pool.tile([128, CH], BF16, name=f"vt{t}", tag="vt")
                qmap[t].dma_start(vt[:], v128[:, t * CH : (t + 1) * CH])
                vts.append(vt)

            pm_acc = small_pool.tile([128, NCH], F32, name="pm_acc")

            # squares: Act takes chunks 0..6 (Square+accum), DVE takes 7
            for t in range(7):
                sq = scrA_pool.tile([128, CH], BF16, name="sq", tag="sq")
                nc.scalar.activation(
                    sq[:], vts[t][:], AF.Square,
                    accum_out=pm_acc[:, t : t + 1],
                )

            # plain sum: DVE pairwise tree (tensor_tensor 2x bf16) + one reduce
            l1 = []
            for k in range(4):
                tt = tree_pool.tile([128, CH], BF16, name=f"l1_{k}", tag="tr")
                nc.vector.tensor_tensor(tt[:], vts[2 * k][:], vts[2 * k + 1][:], ALU.add)
                l1.append(tt)
            l2 = []
            for k in range(2):
                tt = tree_pool.tile([128, CH], BF16, name=f"l2_{k}", tag=f"l2{k}",
                                    bufs=1)
                nc.vector.tensor_tensor(tt[:], l1[2 * k][:], l1[2 * k + 1][:], ALU.add)
                l2.append(tt)
            l3 = tree_pool.tile([128, CH], BF16, name="l3", tag="l3", bufs=1)
            nc.vector.tensor_tensor(l3[:], l2[0][:], l2[1][:], ALU.add)
            # fold l3 down to [128, 256] with 2x tensor_tensor halves; the
            # host does the final column sums (same glue class as the
            # partition pair-add). sqd7 reuses an l2 buffer: the WAR keeps the
            # scheduler from running it before l3 consumed the l2 tiles.
            f1 = tree_pool.tile([128, CH // 2], BF16, name="f1", tag="l20", bufs=1)
            nc.vector.tensor_tensor(
                f1[:], l3[:, 0 : CH // 2], l3[:, CH // 2 : CH], ALU.add
            )
            sqd = tree_pool.tile([128, CH], BF16, name="sqd7", tag="l21", bufs=1)
            nc.vector.scalar_tensor_tensor(
                sqd[:], vts[7][:], 1.0, vts[7][:],
                op0=ALU.mult, op1=ALU.mult,
                accum_out=pm_acc[:, 7:8],
            )
            nc.sync.dma_start(out_pm[:], pm_acc[:])
            nc.sync.dma_start(out_ms[:], f1[:])

    nc.compile()
    return nc


def build_kernel_b():
    """Main kernel: build G, two bf16 matmul chains, write transposed slabs."""
    nc = bacc.Bacc("TRN2", target_bir_lowering=False, debug=False, num_devices=NCORES)

    # pmv: cols 0:4 = a[128*it+p], 4:8 = sp[128*it+p], 8:12 = m2[128*jt+p]
    pmv = nc.dram_tensor("pmv", [128, 3 * KT], F32, kind="ExternalInput")
    m2bc = nc.dram_tensor("m2bc", [128, D], BF16, kind="ExternalInput")
    maskt = nc.dram_tensor("maskt", [128, KT * D], BF16, kind="ExternalInput")
    # ec2[h][p, k*512 + (0:256|256:512)] = sp_j*Ec[j,:] | a_j*Ec[j,:], j=128*(2h+k)+p
    ec2 = nc.dram_tensor("ec2", [2, 128, 2 * 2 * COLS_PER_CORE], BF16,
                         kind="ExternalInput")
    # e2[h][p, k*S + s] = E[128*(2h+k)+p, s]
    e2 = nc.dram_tensor("e2", [2, 128, 2 * S], BF16, kind="ExternalInput")
    out_re = nc.dram_tensor("out_re", [COLS_PER_CORE, S], BF16, kind="ExternalOutput")
    out_im = nc.dram_tensor("out_im", [COLS_PER_CORE, S], BF16, kind="ExternalOutput")

    with tile.TileContext(nc) as tc:
        with (
            tc.tile_pool(name="epool", bufs=1) as e_pool,
            tc.tile_pool(name="small", bufs=1) as small_pool,
            tc.tile_pool(name="gb", bufs=1) as g_pool,
            tc.tile_pool(name="tsb", bufs=1) as t_pool,
            tc.tile_pool(name="ost", bufs=4) as o_pool,
            tc.tile_pool(name="psA", bufs=4, space="PSUM") as psA,
            tc.tile_pool(name="psB", bufs=4, space="PSUM") as psB,
        ):
            # -------- early act-table load + warm-up weights on DVE -----------
            dummy = small_pool.tile([128, 16], BF16, name="dummy")
            nc.vector.memset(dummy[:], 0.5)
            dummy2 = small_pool.tile([128, 16], BF16, name="dummy2")
            nc.scalar.activation(dummy2[:], dummy[:], AF.Abs_reciprocal_sqrt)
            warm_w = small_pool.tile([128, 128], BF16, name="warm_w")
            nc.vector.memset(warm_w[:], 0.001)
            warm_r = small_pool.tile([128, 512], BF16, name="warm_r")
            nc.vector.memset(warm_r[:], 0.001)

            # -------- input DMAs --------------------------------------------
            # sync: smalls + per-jt mask slices, then e2 bulk LAST (big packets
            # would otherwise starve the small gating transfers).
            # scalar: ec2 early (Act queue is free until rinv).
            m2t = small_pool.tile([128, D], BF16, name="m2t")
            nc.sync.dma_start(m2t[:], m2bc[:])
            pv = small_pool.tile([128, 3 * KT], F32, name="pv")
            nc.sync.dma_start(pv[:], pmv[:])
            ec_sb = []
            for h in range(2):
                ect = e_pool.tile([128, 4 * COLS_PER_CORE], BF16, name=f"ec{h}",
                                  tag=f"ec{h}")
                ec_sb.append(ect)
            nc.scalar.dma_start(ec_sb[1][:], ec2[1])
            mk = small_pool.tile([128, KT * D], BF16, name="mk")
            for jt in range(2):
                nc.sync.dma_start(mk[:, jt * D : (jt + 1) * D],
                                  maskt[:, jt * D : (jt + 1) * D])
            nc.sync.dma_start(ec_sb[0][:], ec2[0])
            for jt in range(2, KT):
                nc.sync.dma_start(mk[:, jt * D : (jt + 1) * D],
                                  maskt[:, jt * D : (jt + 1) * D])
            e_sb = []
            for h in range(2):
                et = e_pool.tile([128, 2 * S], BF16, name=f"e{h}", tag=f"e{h}")
                nc.sync.dma_start(et[:], e2[h])
                e_sb.append(et)

            def ec_slice(jt):
                return ec_sb[jt // 2][:, (jt % 2) * 512 : (jt % 2) * 512 + 512]

            def e_slice(it, sn):
                return e_sb[it // 2][:, (it % 2) * S + sn * 512 : (it % 2) * S + (sn + 1) * 512]

            # -------- PE warm-up (ramps the HAM clock before chain1) ----------
            ps_w = psB.tile([128, 512], F32, name="ps_w", tag="o")
            for i in range(WARMUP_MMS):
                nc.tensor.matmul(
                    ps_w[:], warm_w[:], warm_r[:],
                    start=(i == 0), stop=(i == WARMUP_MMS - 1),
                )

            # -------- build G (bf16): rinv on Act, mask-mult on DVE -----------
            g16 = []
            for jt in range(KT):
                rv = g_pool.tile([128, D], BF16, name=f"rv{jt}", tag=f"rv{jt}")
                nc.scalar.activation(
                    rv[:], m2t[:], AF.Abs_reciprocal_sqrt,
                    bias=pv[:, 2 * KT + jt : 2 * KT + jt + 1], scale=1.0,
                )
                gt = g_pool.tile([128, D], BF16, name=f"g{jt}", tag=f"g{jt}")
                nc.vector.tensor_tensor(
                    gt[:], rv[:], mk[:, jt * D : (jt + 1) * D], ALU.mult
                )
                g16.append(gt)

            # -------- chain1: ps_t[it] = sum_jt g16[jt][:,it]^T @ ecs[jt] -----
            ps_ts = [
                psA.tile([128, 2 * COLS_PER_CORE], F32, name=f"ps_t{it}",
                         tag=f"t{it}", bufs=1)
                for it in range(KT)
            ]
            for jt in range(KT):
                for it in range(KT):
                    nc.tensor.matmul(
                        ps_ts[it][:],
                        g16[jt][:, it * 128 : (it + 1) * 128],
                        ec_slice(jt),
                        start=(jt == 0), stop=(jt == KT - 1),
                    )

            # -------- T copies: re-half x a_i (Act), im-half x sp_i (DVE) -----
            t_sb = []
            for it in range(KT):
                tsb = t_pool.tile(
                    [128, 2 * COLS_PER_CORE], BF16, name=f"tsb{it}", tag=f"tsb{it}"
                )
                nc.scalar.activation(
                    tsb[:, 0:COLS_PER_CORE], ps_ts[it][:, 0:COLS_PER_CORE],
                    AF.Copy, scale=pv[:, it : it + 1],
                )
                nc.vector.tensor_scalar(
                    tsb[:, COLS_PER_CORE : 2 * COLS_PER_CORE],
                    ps_ts[it][:, COLS_PER_CORE : 2 * COLS_PER_CORE],
                    pv[:, KT + it : KT + it + 1], None, op0=ALU.mult,
                )
                t_sb.append(tsb)

            # -------- chain2: outT[c,:] = sum_it t16[it][:,c]^T @ e16[it] -----
            # pso groups alternate between the two PSUM pools (8 banks total);
            # outputs are staged into one [128, 2048] tile per group -> 1 DMA.
            NS = S // 512
            for gi, (part, outT, mc) in enumerate(
                ((0, out_re, 0), (0, out_re, 1), (1, out_im, 0), (1, out_im, 1))
            ):
                c0 = part * COLS_PER_CORE + mc * 128
                pool = psB if gi % 2 == 0 else psA
                pso = [
                    pool.tile([128, 512], F32, name=f"pso{gi}_{sn}",
                              tag=("o" if gi % 2 == 0 else f"t{sn}"),
                              bufs=(4 if gi % 2 == 0 else 1))
                    for sn in range(NS)
                ]
                for it in range(KT):
                    for sn in range(NS):
                        nc.tensor.matmul(
                            pso[sn][:],
                            t_sb[it][:, c0 : c0 + 128],
                            e_slice(it, sn),
                            start=(it == 0), stop=(it == KT - 1),
                        )
                osb = o_pool.tile([128, S], BF16, name=f"osb{gi}", tag="osb")
                if gi < 3:
                    for sn in range(NS):
                        dst = osb[:, sn * 512 : (sn + 1) * 512]
                        if gi % 2 == 0:
                            nc.scalar.activation(dst, pso[sn][:], AF.Copy)
                        else:
                            nc.vector.tensor_scalar(
                                dst, pso[sn][:], 1.0, None, op0=ALU.mult
                            )
                    eng = nc.sync if gi % 2 == 0 else nc.scalar
                    eng.dma_start(outT[mc * 128 : (mc + 1) * 128, :], osb[:])
                else:
                    # tail group: both engines copy, two half DMAs overlap
                    for sn in range(2):
                        nc.scalar.activation(
                            osb[:, sn * 512 : (sn + 1) * 512], pso[sn][:], AF.Copy
                        )
                    nc.sync.dma_start(
                        outT[mc * 128 : (mc + 1) * 128, 0:1024], osb[:, 0:1024]
                    )
                    for sn in range(2, NS):
                        nc.vector.tensor_scalar(
                            osb[:, sn * 512 : (sn + 1) * 512], pso[sn][:],
                            1.0, None, op0=ALU.mult,
                        )
                    nc.scalar.dma_start(
                        outT[mc * 128 : (mc + 1) * 128, 1024:2048],
                        osb[:, 1024:2048],
                    )

    nc.compile()
    return nc


def _prepare_a_in_maps(vulns):
    vulns = np.ascontiguousarray(np.asarray(vulns, dtype=np.float32))
    v16 = vulns.astype(NP_BF16)
    in_maps = []
    for c in range(NCORES):
        vsh = v16[c * ROWS_PER_CORE : (c + 1) * ROWS_PER_CORE]
        in_maps.append({"v128": np.ascontiguousarray(vsh.reshape(128, VPART))})
    return in_maps


def _prepare_b_in_maps(embed_table, domain_ids, p_full, msum_full):
    embed_table = np.ascontiguousarray(np.asarray(embed_table, dtype=np.float32))
    domain_ids = np.asarray(domain_ids).astype(np.int64)
    E = np.ascontiguousarray(embed_table[domain_ids])  # [512, 2048] f32

    p = p_full.astype(np.float64)
    m = msum_full.astype(np.float64) / V
    sp = np.sqrt(p)
    a = m * sp
    m2 = (m * m).astype(np.float32)

    pmv = np.empty((128, 3 * KT), dtype=np.float32)
    pmv[:, 0:KT] = a.astype(np.float32).reshape(KT, 128).T
    pmv[:, KT : 2 * KT] = sp.astype(np.float32).reshape(KT, 128).T
    pmv[:, 2 * KT : 3 * KT] = m2.reshape(KT, 128).T

    m2bc = np.ascontiguousarray(
        np.broadcast_to(m2.astype(NP_BF16), (128, D))
    )
    # maskt[p, jt*D + i] = 1.0 iff i < 128*jt + p   (strictly-upper C in [j,i])
    i_idx = np.arange(D)[None, :]
    maskt = np.empty((128, KT * D), dtype=NP_BF16)
    pcol = np.arange(128)[:, None]
    for jt in range(KT):
        maskt[:, jt * D : (jt + 1) * D] = (i_idx < (128 * jt + pcol)).astype(NP_BF16)

    # e2[h][p, k*S + s] = E[128*(2h+k)+p, s]
    e4 = E.astype(NP_BF16).reshape(2, 2, 128, S)  # [h, k, p, s]
    e2_arr = np.ascontiguousarray(e4.transpose(0, 2, 1, 3).reshape(2, 128, 2 * S))

    sp_col = sp.astype(np.float32)[:, None]
    a_col = a.astype(np.float32)[:, None]

    in_maps = []
    for c in range(NCORES):
        Ec = E[:, c * COLS_PER_CORE : (c + 1) * COLS_PER_CORE]
        spEc = (sp_col * Ec).astype(NP_BF16).reshape(KT, 128, COLS_PER_CORE)
        aEc = (a_col * Ec).astype(NP_BF16).reshape(KT, 128, COLS_PER_CORE)
        # ecj[jt][p, 0:256|256:512]
        ecj = np.empty((KT, 128, 2 * COLS_PER_CORE), dtype=NP_BF16)
        ecj[:, :, 0:COLS_PER_CORE] = spEc
        ecj[:, :, COLS_PER_CORE:] = aEc
        # ec2[h][p, k*512 + c2] = ecj[2h+k][p, c2]
        ec2_arr = np.ascontiguousarray(
            ecj.reshape(2, 2, 128, 2 * COLS_PER_CORE)
            .transpose(0, 2, 1, 3)
            .reshape(2, 128, 4 * COLS_PER_CORE)
        )
        in_maps.append(
            {
                "pmv": pmv,
                "m2bc": m2bc,
                "maskt": maskt,
                "ec2": ec2_arr,
                "e2": e2_arr,
            }
        )
    return in_maps


def kernel(vulns, embed_table, domain_ids, _trace=False):
    if "nc_a" not in _CACHE:
        _CACHE["nc_a"] = build_kernel_a()
    if "nc_b" not in _CACHE:
        _CACHE["nc_b"] = build_kernel_b()

    res_a = run_bass_kernel_spmd(
        _CACHE["nc_a"], _prepare_a_in_maps(vulns),
        core_ids=list(range(NCORES)), trace=_trace,
    )
    _CACHE["res_a"] = res_a
    pp = np.concatenate(
        [np.asarray(res_a.results[c]["out_pm"], np.float32).sum(axis=1)
         for c in range(NCORES)]
    )  # [1024] per-partition p
    ms = np.concatenate(
        [np.asarray(res_a.results[c]["out_ms"], np.float32).sum(axis=1)
         for c in range(NCORES)]
    )  # [1024] per-partition msum
    p_full = pp.reshape(D, 2).sum(axis=1)
    msum_full = ms.reshape(D, 2).sum(axis=1)

    res_b = run_bass_kernel_spmd(
        _CACHE["nc_b"], _prepare_b_in_maps(embed_table, domain_ids, p_full, msum_full),
        core_ids=list(range(NCORES)), trace=_trace,
    )
    _CACHE["res_b"] = res_b

    out = np.empty((S, S), dtype=np.complex64)
    for c in range(NCORES):
        r = res_b.results[c]
        sl = slice(c * COLS_PER_CORE, (c + 1) * COLS_PER_CORE)
        re = np.asarray(r["out_re"], dtype=np.float32)
        im = np.asarray(r["out_im"], dtype=np.float32)
        out[:, sl] = re.T + 1j * im.T
    return out


if __name__ == "__main__":
    rng = np.random.default_rng(0)
    v = rng.standard_normal((D, V), dtype=np.float32)
    et = rng.standard_normal((D, S), dtype=np.float32)
    ids = np.arange(D, dtype=np.int32)
    out = kernel(v, et, ids)
    print(out.shape, out.dtype)


# revision 21
# speedup vs baseline: 1.0985x; 1.0985x over previous
"""Trainium2 Bass kernel for nn_OmegaEntangle (E^T C E with entangle coefficients).

Math (validated vs reference to ~5.3e-3 rel err in the numpy bf16 model):
  p_i = sum_j v_ij^2 ; m_i = mean_j v_ij
  C[i,j] = mask(i<j) * sqrt(p_i p_j) * (m_i + 1j*m_j) / sqrt(m_i^2 + m_j^2)
  out = E^T C E   (complex, E real)

Factorization used on device (amp factors folded into operand scaling):
  G[i,j]  = mask(i<j) / sqrt(m_i^2 + m_j^2)          (the only matrix built on-chip)
  T_re    = diag(a) G (diag(sp) E) ;  T_im = diag(sp) G (diag(a) E)
  out_re  = E^T T_re ; out_im = E^T T_im             (a = m*sqrt(p), sp = sqrt(p))

Sharding: data-parallel over the 2048 OUTPUT COLUMNS (256 per core), with the
p/m reduction row-sharded (64 rows per core => 128 SBUF partitions).

Two NEFF launches (host concat of the tiny reduction result between them):
  Kernel A: 2-engine reduce of the [128, 16384] bf16 vuln shard:
    Act does Square+accum on 7 chunks; DVE squares 1 chunk and computes the
    plain sum via a pairwise tensor_tensor tree (2x bf16) + one tensor_reduce.
  Kernel B: build G via Abs_reciprocal_sqrt + mask, two bf16 matmul chains,
    write transposed [256, 2048] bf16 slabs for re/im.
DMA layouts use >=4KB per-partition contiguous runs (small packets starve the
DMA engines). Only HWDGE queues (sync/scalar) carry bulk data; the gating small
tensors are ordered ahead of the 2MB E stream so they are not starved, and the
e2 bulk rides behind the masks on sync. PE warm-up fillers run before chain1 so
the HAM clock reaches 2.4GHz without a >3.4us idle re-throttle.
"""

import numpy as np
import ml_dtypes

import concourse.bass as bass
import concourse.mybir as mybir
import concourse.tile as tile
from concourse import bacc
from concourse.bass_utils import run_bass_kernel_spmd

D = 512          # number of domains
V = 32768        # vuln dim
S = 2048         # sup (embed) dim
NCORES = 8
ROWS_PER_CORE = D // NCORES          # 64
COLS_PER_CORE = S // NCORES          # 256
KT = D // 128                         # 4 contraction tiles
VPART = (ROWS_PER_CORE * V) // 128    # 16384 vuln elems per partition
NCH = 8                               # reduce chunks per core
CH = VPART // NCH                     # 2048
WARMUP_MMS = 5

F32 = mybir.dt.float32
BF16 = mybir.dt.bfloat16
NP_BF16 = ml_dtypes.bfloat16
AF = mybir.ActivationFunctionType
ALU = mybir.AluOpType

_CACHE = {}


def build_kernel_a():
    """Reduce kernel: per-partition p/msum over the [128, 16384] bf16 shard."""
    nc = bacc.Bacc("TRN2", target_bir_lowering=False, debug=False, num_devices=NCORES)

    v128 = nc.dram_tensor("v128", [128, VPART], BF16, kind="ExternalInput")
    out_pm = nc.dram_tensor("out_pm", [128, NCH], F32, kind="ExternalOutput")
    out_ms = nc.dram_tensor("out_ms", [128, CH // 2], BF16,
                            kind="ExternalOutput")

    with tile.TileContext(nc) as tc:
        with (
            tc.tile_pool(name="vin", bufs=NCH) as vin_pool,
            tc.tile_pool(name="scrA", bufs=3) as scrA_pool,
            tc.tile_pool(name="scrD", bufs=2) as scrD_pool,
            tc.tile_pool(name="tree", bufs=4) as tree_pool,
            tc.tile_pool(name="small", bufs=1) as small_pool,
        ):
            vts = []
            qmap = {0: nc.sync, 1: nc.scalar, 2: nc.sync, 3: nc.scalar,
                    4: nc.sync, 5: nc.scalar, 6: nc.sync, 7: nc.scalar}
            for t in range(NCH):
                vt = vin_pool.tile([128, CH], BF16, name=f"vt{t}", tag="vt")
                qmap[t].dma_start(vt[:], v128[:, t * CH : (t + 1) * CH])
                vts.append(vt)

            pm_acc = small_pool.tile([128, NCH], F32, name="pm_acc")

            # squares: Act takes chunks 0..6 (Square+accum), DVE takes 7
            for t in range(7):
                sq = scrA_pool.tile([128, CH], BF16, name="sq", tag="sq")
                nc.scalar.activation(
                    sq[:], vts[t][:], AF.Square,
                    accum_out=pm_acc[:, t : t + 1],
                )

            # plain sum: DVE pairwise tree (tensor_tensor 2x bf16) + one reduce
            l1 = []
            for k in range(4):
                tt = tree_pool.tile([128, CH], BF16, name=f"l1_{k}", tag="tr")
                nc.vector.tensor_tensor(tt[:], vts[2 * k][:], vts[2 * k + 1][:], ALU.add)
                l1.append(tt)
            l2 = []
            for k in range(2):
                tt = tree_pool.tile([128, CH], BF16, name=f"l2_{k}", tag=f"l2{k}",
                                    bufs=1)
                nc.vector.tensor_tensor(tt[:], l1[2 * k][:], l1[2 * k + 1][:], ALU.add)
                l2.append(tt)
            l3 = tree_pool.tile([128, CH], BF16, name="l3", tag="l3", bufs=1)
            nc.vector.tensor_tensor(l3[:], l2[0][:], l2[1][:], ALU.add)
            # fold l3 down to [128, 256] with 2x tensor_tensor halves; the
            # host does the final column sums (same glue class as the
            # partition pair-add). sqd7 reuses an l2 buffer: the WAR keeps the
            # scheduler from running it before l3 consumed the l2 tiles.
            f1 = tree_pool.tile([128, CH // 2], BF16, name="f1", tag="l20", bufs=1)
            nc.vector.tensor_tensor(
                f1[:], l3[:, 0 : CH // 2], l3[:, CH // 2 : CH], ALU.add
            )
            sqd = tree_pool.tile([128, CH], BF16, name="sqd7", tag="l21", bufs=1)
            nc.vector.scalar_tensor_tensor(
                sqd[:], vts[7][:], 1.0, vts[7][:],
                op0=ALU.mult, op1=ALU.mult,
                accum_out=pm_acc[:, 7:8],
            )
            nc.sync.dma_start(out_pm[:], pm_acc[:])
            nc.sync.dma_start(out_ms[:], f1[:])

    nc.compile()
    return nc


def build_kernel_b():
    """Main kernel: build G, two bf16 matmul chains, write transposed slabs."""
    nc = bacc.Bacc("TRN2", target_bir_lowering=False, debug=False, num_devices=NCORES)

    # pmv: cols 0:4 = a[128*it+p], 4:8 = sp[128*it+p], 8:12 = m2[128*jt+p]
    pmv = nc.dram_tensor("pmv", [128, 3 * KT], F32, kind="ExternalInput")
    m2bc = nc.dram_tensor("m2bc", [128, D], BF16, kind="ExternalInput")
    # ec2[h][p, k*512 + (0:256|256:512)] = sp_j*Ec[j,:] | a_j*Ec[j,:], j=128*(2h+k)+p
    ec2 = nc.dram_tensor("ec2", [2, 128, 2 * 2 * COLS_PER_CORE], BF16,
                         kind="ExternalInput")
    # e2[h][p, k*S + s] = E[128*(2h+k)+p, s]
    e2 = nc.dram_tensor("e2", [2, 128, 2 * S], BF16, kind="ExternalInput")
    out_re = nc.dram_tensor("out_re", [COLS_PER_CORE, S], BF16, kind="ExternalOutput")
    out_im = nc.dram_tensor("out_im", [COLS_PER_CORE, S], BF16, kind="ExternalOutput")

    with tile.TileContext(nc) as tc:
        with (
            tc.tile_pool(name="epool", bufs=1) as e_pool,
            tc.tile_pool(name="small", bufs=1) as small_pool,
            tc.tile_pool(name="gb", bufs=1) as g_pool,
            tc.tile_pool(name="tsb", bufs=1) as t_pool,
            tc.tile_pool(name="ost", bufs=4) as o_pool,
            tc.tile_pool(name="psA", bufs=4, space="PSUM") as psA,
            tc.tile_pool(name="psB", bufs=4, space="PSUM") as psB,
        ):
            # -------- early act-table load + warm-up weights on DVE -----------
            dummy = small_pool.tile([128, 16], BF16, name="dummy")
            nc.vector.memset(dummy[:], 0.5)
            dummy2 = small_pool.tile([128, 16], BF16, name="dummy2")
            nc.scalar.activation(dummy2[:], dummy[:], AF.Abs_reciprocal_sqrt)
            warm_w = small_pool.tile([128, 128], BF16, name="warm_w")
            nc.vector.memset(warm_w[:], 0.001)
            warm_r = small_pool.tile([128, 512], BF16, name="warm_r")
            nc.vector.memset(warm_r[:], 0.001)

            # -------- input DMAs --------------------------------------------
            # sync: smalls + per-jt mask slices, then e2 bulk LAST (big packets
            # would otherwise starve the small gating transfers).
            # scalar: ec2 early (Act queue is free until rinv).
            m2t = small_pool.tile([128, D], BF16, name="m2t")
            nc.sync.dma_start(m2t[:], m2bc[:])
            pv = small_pool.tile([128, 3 * KT], F32, name="pv")
            nc.sync.dma_start(pv[:], pmv[:])
            ec_sb = []
            for h in range(2):
                ect = e_pool.tile([128, 4 * COLS_PER_CORE], BF16, name=f"ec{h}",
                                  tag=f"ec{h}")
                ec_sb.append(ect)
            nc.scalar.dma_start(ec_sb[1][:], ec2[1])
            nc.sync.dma_start(ec_sb[0][:], ec2[0])
            e_sb = []
            for h in range(2):
                et = e_pool.tile([128, 2 * S], BF16, name=f"e{h}", tag=f"e{h}")
                nc.sync.dma_start(et[:], e2[h])
                e_sb.append(et)

            def ec_slice(jt):
                return ec_sb[jt // 2][:, (jt % 2) * 512 : (jt % 2) * 512 + 512]

            def e_slice(it, sn):
                return e_sb[it // 2][:, (it % 2) * S + sn * 512 : (it % 2) * S + (sn + 1) * 512]

            # -------- PE warm-up (ramps the HAM clock before chain1) ----------
            ps_w = psB.tile([128, 512], F32, name="ps_w", tag="o")
            for i in range(WARMUP_MMS):
                nc.tensor.matmul(
                    ps_w[:], warm_w[:], warm_r[:],
                    start=(i == 0), stop=(i == WARMUP_MMS - 1),
                )

            # -------- build G (bf16): rinv on Act, mask-mult on DVE -----------
            g16 = []
            for jt in range(KT):
                rv = g_pool.tile([128, D], BF16, name=f"rv{jt}", tag=f"rv{jt}")
                nc.scalar.activation(
                    rv[:], m2t[:], AF.Abs_reciprocal_sqrt,
                    bias=pv[:, 2 * KT + jt : 2 * KT + jt + 1], scale=1.0,
                )
                gt = g_pool.tile([128, D], BF16, name=f"g{jt}", tag=f"g{jt}")
                nc.gpsimd.affine_select(
                    out=gt[:], in_=rv[:],
                    pattern=[[-1, D]], compare_op=ALU.is_gt,
                    fill=0.0, base=128 * jt, channel_multiplier=1,
                )
                g16.append(gt)

            # -------- chain1: ps_t[it] = sum_jt g16[jt][:,it]^T @ ecs[jt] -----
            ps_ts = [
                psA.tile([128, 2 * COLS_PER_CORE], F32, name=f"ps_t{it}",
                         tag=f"t{it}", bufs=1)
                for it in range(KT)
            ]
            for jt in range(KT):
                for it in range(KT):
                    nc.tensor.matmul(
                        ps_ts[it][:],
                        g16[jt][:, it * 128 : (it + 1) * 128],
                        ec_slice(jt),
                        start=(jt == 0), stop=(jt == KT - 1),
                    )

            # -------- T copies: re-half x a_i (Act), im-half x sp_i (DVE) -----
            t_sb = []
            for it in range(KT):
                tsb = t_pool.tile(
                    [128, 2 * COLS_PER_CORE], BF16, name=f"tsb{it}", tag=f"tsb{it}"
                )
                nc.scalar.activation(
                    tsb[:, 0:COLS_PER_CORE], ps_ts[it][:, 0:COLS_PER_CORE],
                    AF.Copy, scale=pv[:, it : it + 1],
                )
                nc.vector.tensor_scalar(
                    tsb[:, COLS_PER_CORE : 2 * COLS_PER_CORE],
                    ps_ts[it][:, COLS_PER_CORE : 2 * COLS_PER_CORE],
                    pv[:, KT + it : KT + it + 1], None, op0=ALU.mult,
                )
                t_sb.append(tsb)

            # -------- chain2: outT[c,:] = sum_it t16[it][:,c]^T @ e16[it] -----
            # pso groups alternate between the two PSUM pools (8 banks total);
            # outputs are staged into one [128, 2048] tile per group -> 1 DMA.
            NS = S // 512
            for gi, (part, outT, mc) in enumerate(
                ((0, out_re, 0), (0, out_re, 1), (1, out_im, 0), (1, out_im, 1))
            ):
                c0 = part * COLS_PER_CORE + mc * 128
                pool = psB if gi % 2 == 0 else psA
                pso = [
                    pool.tile([128, 512], F32, name=f"pso{gi}_{sn}",
                              tag=("o" if gi % 2 == 0 else f"t{sn}"),
                              bufs=(4 if gi % 2 == 0 else 1))
                    for sn in range(NS)
                ]
                for it in range(KT):
                    for sn in range(NS):
                        nc.tensor.matmul(
                            pso[sn][:],
                            t_sb[it][:, c0 : c0 + 128],
                            e_slice(it, sn),
                            start=(it == 0), stop=(it == KT - 1),
                        )
                osb = o_pool.tile([128, S], BF16, name=f"osb{gi}", tag="osb")
                if gi < 3:
                    for sn in range(NS):
                        dst = osb[:, sn * 512 : (sn + 1) * 512]
                        if gi % 2 == 0:
                            nc.scalar.activation(dst, pso[sn][:], AF.Copy)
                        else:
                            nc.vector.tensor_scalar(
                                dst, pso[sn][:], 1.0, None, op0=ALU.mult
                            )
                    eng = nc.sync if gi % 2 == 0 else nc.scalar
                    eng.dma_start(outT[mc * 128 : (mc + 1) * 128, :], osb[:])
                else:
                    # tail group: both engines copy, two half DMAs overlap
                    for sn in range(2):
                        nc.scalar.activation(
                            osb[:, sn * 512 : (sn + 1) * 512], pso[sn][:], AF.Copy
                        )
                    nc.sync.dma_start(
                        outT[mc * 128 : (mc + 1) * 128, 0:1024], osb[:, 0:1024]
                    )
                    for sn in range(2, NS):
                        nc.vector.tensor_scalar(
                            osb[:, sn * 512 : (sn + 1) * 512], pso[sn][:],
                            1.0, None, op0=ALU.mult,
                        )
                    nc.scalar.dma_start(
                        outT[mc * 128 : (mc + 1) * 128, 1024:2048],
                        osb[:, 1024:2048],
                    )

    nc.compile()
    return nc


def _prepare_a_in_maps(vulns):
    vulns = np.ascontiguousarray(np.asarray(vulns, dtype=np.float32))
    v16 = vulns.astype(NP_BF16)
    in_maps = []
    for c in range(NCORES):
        vsh = v16[c * ROWS_PER_CORE : (c + 1) * ROWS_PER_CORE]
        in_maps.append({"v128": np.ascontiguousarray(vsh.reshape(128, VPART))})
    return in_maps


def _prepare_b_in_maps(embed_table, domain_ids, p_full, msum_full):
    embed_table = np.ascontiguousarray(np.asarray(embed_table, dtype=np.float32))
    domain_ids = np.asarray(domain_ids).astype(np.int64)
    E = np.ascontiguousarray(embed_table[domain_ids])  # [512, 2048] f32

    p = p_full.astype(np.float64)
    m = msum_full.astype(np.float64) / V
    sp = np.sqrt(p)
    a = m * sp
    m2 = (m * m).astype(np.float32)

    pmv = np.empty((128, 3 * KT), dtype=np.float32)
    pmv[:, 0:KT] = a.astype(np.float32).reshape(KT, 128).T
    pmv[:, KT : 2 * KT] = sp.astype(np.float32).reshape(KT, 128).T
    pmv[:, 2 * KT : 3 * KT] = m2.reshape(KT, 128).T

    m2bc = np.ascontiguousarray(
        np.broadcast_to(m2.astype(NP_BF16), (128, D))
    )
    # e2[h][p, k*S + s] = E[128*(2h+k)+p, s]
    e4 = E.astype(NP_BF16).reshape(2, 2, 128, S)  # [h, k, p, s]
    e2_arr = np.ascontiguousarray(e4.transpose(0, 2, 1, 3).reshape(2, 128, 2 * S))

    sp_col = sp.astype(np.float32)[:, None]
    a_col = a.astype(np.float32)[:, None]

    in_maps = []
    for c in range(NCORES):
        Ec = E[:, c * COLS_PER_CORE : (c + 1) * COLS_PER_CORE]
        spEc = (sp_col * Ec).astype(NP_BF16).reshape(KT, 128, COLS_PER_CORE)
        aEc = (a_col * Ec).astype(NP_BF16).reshape(KT, 128, COLS_PER_CORE)
        # ecj[jt][p, 0:256|256:512]
        ecj = np.empty((KT, 128, 2 * COLS_PER_CORE), dtype=NP_BF16)
        ecj[:, :, 0:COLS_PER_CORE] = spEc
        ecj[:, :, COLS_PER_CORE:] = aEc
        # ec2[h][p, k*512 + c2] = ecj[2h+k][p, c2]
        ec2_arr = np.ascontiguousarray(
            ecj.reshape(2, 2, 128, 2 * COLS_PER_CORE)
            .transpose(0, 2, 1, 3)
            .reshape(2, 128, 4 * COLS_PER_CORE)
        )
        in_maps.append(
            {
                "pmv": pmv,
                "m2bc": m2bc,
                "ec2": ec2_arr,
                "e2": e2_arr,
            }
        )
    return in_maps


def kernel(vulns, embed_table, domain_ids, _trace=False):
    if "nc_a" not in _CACHE:
        _CACHE["nc_a"] = build_kernel_a()
    if "nc_b" not in _CACHE:
        _CACHE["nc_b"] = build_kernel_b()

    res_a = run_bass_kernel_spmd(
        _CACHE["nc_a"], _prepare_a_in_maps(vulns),
        core_ids=list(range(NCORES)), trace=_trace,
    )
    _CACHE["res_a"] = res_a
    pp = np.concatenate(
        [np.asarray(res_a.results[c]["out_pm"], np.float32).sum(axis=1)
         for c in range(NCORES)]
    )  # [1024] per-partition p
    ms = np.concatenate(
        [np.asarray(res_a.results[c]["out_ms"], np.float32).sum(axis=1)
         for c in range(NCORES)]
    )  # [1024] per-partition msum
    p_full = pp.reshape(D, 2).sum(axis=1)
    msum_full = ms.reshape(D, 2).sum(axis=1)

    res_b = run_bass_kernel_spmd(
        _CACHE["nc_b"], _prepare_b_in_maps(embed_table, domain_ids, p_full, msum_full),
        core_ids=list(range(NCORES)), trace=_trace,
    )
    _CACHE["res_b"] = res_b

    out = np.empty((S, S), dtype=np.complex64)
    for c in range(NCORES):
        r = res_b.results[c]
        sl = slice(c * COLS_PER_CORE, (c + 1) * COLS_PER_CORE)
        re = np.asarray(r["out_re"], dtype=np.float32)
        im = np.asarray(r["out_im"], dtype=np.float32)
        out[:, sl] = re.T + 1j * im.T
    return out


if __name__ == "__main__":
    rng = np.random.default_rng(0)
    v = rng.standard_normal((D, V), dtype=np.float32)
    et = rng.standard_normal((D, S), dtype=np.float32)
    ids = np.arange(D, dtype=np.int32)
    out = kernel(v, et, ids)
    print(out.shape, out.dtype)
